# revision 44
# baseline (speedup 1.0000x reference)
"""FP8Linear Trainium2 kernel.

Computes out = quant_e4m3(x) @ quant_e4m3(w).T in fp32, distributed over 8
NeuronCores as a 2x4 grid (x rows x w rows). Per core:

  x_in [128, 16, 4096] bf16, w_in [128, 16, 2048] bf16 -> out [4096, 2048] f32

Host-side staging (layout + lossless re-encoding, exact):
  - operands transposed AND pre-tiled to [partition, c-chunk, free] so the
    contraction dim c is the on-chip partition dim and a single DMA
    instruction can load many c-chunks as one natural 3D slice (per-DMA
    fixed cost was the fill-phase bottleneck at 176 small loads; this
    needs only ~22);
  - f32 -> bf16 with ROUND-TO-ODD (truncate + sticky bit into the lsb).
    RNE(bf16_RO(v) -> e4m3) == RNE(v -> e4m3) exactly (double-rounding is
    exact when the intermediate format has >= 2 more mantissa bits than
    the target: bf16 has 8, e4m3 needs 3+2), so the device's quantize
    reproduces the reference f32->fp8 bit-for-bit while input DMA
    traffic halves.

Device pipeline:
  loads: bf16 c-chunk-quad (chase phase) or full-wave (steady state)
     slices -> ACT/DVE quantize bf16->fp8e4 (the rounding step) into
     resident fp8 tiles XT [128c, 16s, 4096m], WT [128c, 16s, 2048o]
  matmul: fp8 DoubleRow (c-chunk pairs on partitions), fp32 PSUM, N=512
     tiles, 8 PSUM banks, DVE drains, 512 KiB f32 output DMAs.

Schedule: w's first o-half + x group 0 load as c-ascending quads that the
first matmul group chases sp-pair by sp-pair; w's second o-half trickles
behind x groups 2-3; the o-half-1 matmul groups are emitted late, when
everything they need is already resident. Loads ride the two HWDGE FIFOs
(sync + scalar), stores ride the SWDGE queue; steady-state psum drains
stay exclusively on DVE so nothing can delay a drain (a blocked drain
holds a PSUM bank and stalls PE).
"""

import numpy as np
import ml_dtypes

# ---- problem constants (hardcoded per task contract) ----
A_DIM, B_DIM, C_DIM, OUT_DIM = 4, 2048, 2048, 8192
M_FULL = A_DIM * B_DIM  # 8192
GRID_M, GRID_O = 2, 4
N_CORES = GRID_M * GRID_O
M_CORE = M_FULL // GRID_M   # 4096
O_CORE = OUT_DIM // GRID_O  # 2048

P = 128
S_CHUNKS = C_DIM // P  # 16


def build_nc(m_core=M_CORE, o_core=O_CORE, c_dim=C_DIM,
             m_slab=512, n_tile=512, mm_psum_bufs=8):
    """Build the single-core Bass program (same program runs SPMD on 8 cores)."""
    import contextlib

    import concourse.bacc as bacc
    import concourse.mybir as mybir
    import concourse.tile as tile

    bf16 = mybir.dt.bfloat16
    f32 = mybir.dt.float32
    fp8 = mybir.dt.float8e4
    Copy = mybir.ActivationFunctionType.Copy
    DR = mybir.MatmulPerfMode.DoubleRow

    S = c_dim // P              # c-chunks (16)
    SP = S // 2                 # DoubleRow pairs (8)
    Q = S // 4                  # c-chunk quads (4)
    MG = m_core // m_slab       # x slab groups (8)
    MWG = m_slab // P           # m windows per slab group (4)
    NT = o_core // n_tile       # o tiles (4)
    NTH = NT // 2               # o tiles per half (2)
    o_half = o_core // 2        # 1024

    nc = bacc.Bacc(None, target_bir_lowering=False, debug=False)
    x_in = nc.declare_dram_parameter("x_in", [P, S, m_core], bf16, isOutput=False)
    w_in = nc.declare_dram_parameter("w_in", [P, S, o_core], bf16, isOutput=False)
    out = nc.declare_dram_parameter("out", [m_core, o_core], f32, isOutput=True)

    with tile.TileContext(nc) as tc:
        with contextlib.ExitStack() as ctx:
            wq = ctx.enter_context(tc.tile_pool(name="wq", bufs=3))
            xq = ctx.enter_context(tc.tile_pool(name="xq", bufs=8))
            xres = ctx.enter_context(tc.tile_pool(name="xres", bufs=1))
            wres = ctx.enter_context(tc.tile_pool(name="wres", bufs=1))
            mmp = ctx.enter_context(
                tc.tile_pool(name="mmp", bufs=mm_psum_bufs, space="PSUM"))
            osb = ctx.enter_context(tc.tile_pool(name="osb", bufs=4))

            # resident fp8 operands, c on partitions
            XT = xres.tile([P, S, m_core], fp8)
            WT = wres.tile([P, S, o_core], fp8)

            def w_seg(c0, nch, oh, qsel, qeng):
                """w c-chunks [c0..c0+nch) x o-half oh: one load + quant."""
                o0 = oh * o_half
                wst = wq.tile([P, 4, o_half], bf16, tag="wq", name="wq")
                src = w_in[:, c0:c0 + nch, o0:o0 + o_half]
                dst_st = wst[:, 0:nch, :]
                if qsel == 0:
                    nc.sync.dma_start(out=dst_st, in_=src)
                else:
                    nc.scalar.dma_start(out=dst_st, in_=src)
                dst = WT[:, c0:c0 + nch, o0:o0 + o_half]
                if qeng == 0:
                    nc.scalar.activation(dst, dst_st, Copy)
                else:
                    nc.vector.tensor_copy(out=dst, in_=dst_st)

            def x_seg(c0, nch, mg, qsel, qeng):
                """x c-chunks [c0..c0+nch) x m slab mg: one load + quant.
                Multi-chunk batching amortizes per-DMA fixed cost while the
                chase stays fine-grained (a chunk pair feeds one sp step)."""
                m0 = mg * m_slab
                xst = xq.tile([P, 4, m_slab], bf16, tag="xq", name="xq")
                src = x_in[:, c0:c0 + nch, m0:m0 + m_slab]
                dst_st = xst[:, 0:nch, :]
                if qsel == 0:
                    nc.sync.dma_start(out=dst_st, in_=src)
                else:
                    nc.scalar.dma_start(out=dst_st, in_=src)
                dst = XT[:, c0:c0 + nch, m0:m0 + m_slab]
                if qeng == 0:
                    nc.scalar.activation(dst, dst_st, Copy)
                else:
                    nc.vector.tensor_copy(out=dst, in_=dst_st)

            def w_quad(q, oh, qsel, qeng):
                w_seg(4 * q, 4, oh, qsel, qeng)

            def x_quad(q, mg, qsel, qeng):
                x_seg(4 * q, 4, mg, qsel, qeng)

            def x_wave(mg, qsel):
                """One x slab group as 4 pipelined quads (load q+1 overlaps
                quant q); a single 2 MiB DMA + monolithic quant was measured
                slower — its serial end-to-end latency stalls the chase."""
                for q in range(Q):
                    x_quad(q, mg, qsel=(q + qsel) % 2, qeng=0)

            def mm_half(mw, oh, tail=False):
                """One 128-row m window x one 1024-col o half: 8 sp x 2 nt
                DoubleRow matmuls, drain 2 psum tiles, 512 KiB store.

                Steady state keeps all drains on DVE (so nothing can delay a
                drain and hold a PSUM bank) and all stores on SWDGE. In the
                tail, ACT and the scalar HWDGE queue are idle, so drains and
                stores spread across both to shorten the endgame."""
                ps = [mmp.tile([P, n_tile], f32, tag="mm_psum", name="mm_psum")
                      for _ in range(NTH)]
                for sp in range(SP):
                    lhsT = XT[:, 2 * sp:2 * sp + 2, mw * P:(mw + 1) * P]
                    for j in range(NTH):
                        nt = NTH * oh + j
                        nc.tensor.matmul(
                            ps[j][:], lhsT,
                            WT[:, 2 * sp:2 * sp + 2, nt * n_tile:(nt + 1) * n_tile],
                            start=(sp == 0), stop=(sp == SP - 1),
                            perf_mode=DR)
                ot = osb.tile([P, o_half], f32, tag="ot", name="ot")
                for j in range(NTH):
                    dst = ot[:, j * n_tile:(j + 1) * n_tile]
                    if tail and j % 2 == 0:
                        nc.scalar.activation(dst, ps[j][:], Copy)
                    else:
                        nc.vector.tensor_copy(out=dst, in_=ps[j][:])
                odst = out[mw * P:(mw + 1) * P, oh * o_half:(oh + 1) * o_half]
                if tail and mw % 2 == 0:
                    nc.scalar.dma_start(out=odst, in_=ot[:])
                else:
                    nc.gpsimd.dma_start(out=odst, in_=ot[:])

            def mg_windows(mg):
                return range(mg * MWG, (mg + 1) * MWG)

            # ---- prefix: segments of (w o-half 0, x group 0) c-ascending on
            # both HWDGE queues, quants split ACT/DVE (safe ONLY here — no
            # drains exist yet); the first two segments are c-PAIRS so the
            # first sp starts ~4us; group 0 chases them sp-pair by sp-pair ----
            segs = [(0, 2), (2, 2), (4, 4), (8, 4), (12, 4)]
            for i, (c0, nch) in enumerate(segs):
                w_seg(c0, nch, 0, qsel=i % 2, qeng=i % 2)
                x_seg(c0, nch, 0, qsel=(i + 1) % 2, qeng=(i + 1) % 2)
            # prefetch wave for mg1 (quads, ACT quants) before group 0's
            # matmuls so its quants process during group 0's stream
            for q in range(Q):
                x_quad(q, 1, qsel=q % 2, qeng=0)
            for mw in mg_windows(0):
                mm_half(mw, 0)

            # ---- w's second o-half trickles behind each wave's x quads ----
            x_wave(2, qsel=0)
            w_quad(0, 1, qsel=0, qeng=1)
            w_quad(1, 1, qsel=1, qeng=1)
            for mw in mg_windows(1):
                mm_half(mw, 0)

            x_wave(3, qsel=1)
            w_quad(2, 1, qsel=0, qeng=1)
            w_quad(3, 1, qsel=1, qeng=1)
            for mw in mg_windows(2):
                mm_half(mw, 0)

            for mg in range(3, MG):
                if mg + 1 < MG:
                    x_wave(mg + 1, qsel=mg % 2)
                for mw in mg_windows(mg):
                    mm_half(mw, 0)
                if mg >= 5:
                    for mw in mg_windows(mg - 5):
                        mm_half(mw, 1)
            for mg in range(MG - 5, MG):
                for mw in mg_windows(mg):
                    mm_half(mw, 1, tail=(mg >= MG - 2))

    nc.finalize()
    return nc


def _round_to_odd_bf16(a):
    """f32 -> bf16 by truncation with the sticky bit ORed into the lsb.

    RNE(result -> e4m3) == RNE(a -> e4m3) exactly (no double rounding).
    """
    u = np.ascontiguousarray(a, dtype=np.float32).view(np.uint32)
    hi = (u >> 16).astype(np.uint16)
    hi |= ((u & 0xFFFF) != 0).astype(np.uint16)
    return hi.view(ml_dtypes.bfloat16)


def _stage(mat_t):
    """[c, free] -> [128, 16, free] with (p, s, f) = mat_t[s*128+p, f]."""
    c, free = mat_t.shape
    return np.ascontiguousarray(
        mat_t.reshape(S_CHUNKS, P, free).transpose(1, 0, 2))


_NC = None


def _get_nc():
    global _NC
    if _NC is None:
        _NC = build_nc()
    return _NC


def kernel(input, weight, input_scale_e4m3=None, weight_scale_e4m3=None,
           **_unused):
    from concourse.bass_utils import run_bass_kernel_spmd

    x = np.asarray(input, dtype=np.float32).reshape(M_FULL, C_DIM)
    w = np.asarray(weight, dtype=np.float32)
    s_in = float(np.asarray(input_scale_e4m3)) if input_scale_e4m3 is not None else 1.0
    s_w = float(np.asarray(weight_scale_e4m3)) if weight_scale_e4m3 is not None else 1.0

    # reference semantics: round(x*s)/s etc.; fold scales on host (exact)
    if s_in != 1.0:
        x = x * s_in
    if s_w != 1.0:
        w = w * s_w

    # host-side staging: round-to-odd bf16 (exact w.r.t. the later fp8
    # RNE quantize), transpose so c is the contraction/partition dim, and
    # pre-tile to [partition, c-chunk, free] for batched multi-chunk DMAs
    xb = _round_to_odd_bf16(x)
    wb = _round_to_odd_bf16(w)
    xT = [_stage(xb[mi * M_CORE:(mi + 1) * M_CORE].T)
          for mi in range(GRID_M)]
    wT = [_stage(wb[oj * O_CORE:(oj + 1) * O_CORE].T)
          for oj in range(GRID_O)]

    nc = _get_nc()
    in_maps = []
    for mi in range(GRID_M):
        for oj in range(GRID_O):
            in_maps.append({"x_in": xT[mi], "w_in": wT[oj]})
    res = run_bass_kernel_spmd(nc, in_maps, core_ids=list(range(N_CORES)))

    out = np.empty((M_FULL, OUT_DIM), np.float32)
    for k, r in enumerate(res.results):
        mi, oj = divmod(k, GRID_O)
        out[mi * M_CORE:(mi + 1) * M_CORE, oj * O_CORE:(oj + 1) * O_CORE] = r["out"]

    inv = 1.0 / (s_in * s_w)
    if inv != 1.0:
        out = out * inv
    return out.reshape(A_DIM, B_DIM, OUT_DIM)


# revision 47
# speedup vs baseline: 1.0263x; 1.0263x over previous
"""FP8Linear Trainium2 kernel.

Computes out = quant_e4m3(x) @ quant_e4m3(w).T in fp32, distributed over 8
NeuronCores as a 2x4 grid (x rows x w rows). Per core:

  x_in [128, 16, 4096] bf16, w_in [128, 16, 2048] bf16 -> out [4096, 2048] f32

Host-side staging (layout + lossless re-encoding, exact):
  - operands transposed AND pre-tiled to [partition, c-chunk, free] so the
    contraction dim c is the on-chip partition dim and a single DMA
    instruction can load many c-chunks as one natural 3D slice (per-DMA
    fixed cost was the fill-phase bottleneck at 176 small loads; this
    needs only ~22);
  - f32 -> bf16 with ROUND-TO-ODD (truncate + sticky bit into the lsb).
    RNE(bf16_RO(v) -> e4m3) == RNE(v -> e4m3) exactly (double-rounding is
    exact when the intermediate format has >= 2 more mantissa bits than
    the target: bf16 has 8, e4m3 needs 3+2), so the device's quantize
    reproduces the reference f32->fp8 bit-for-bit while input DMA
    traffic halves.

Device pipeline:
  loads: bf16 c-chunk-quad (chase phase) or full-wave (steady state)
     slices -> ACT/DVE quantize bf16->fp8e4 (the rounding step) into
     resident fp8 tiles XT [128c, 16s, 4096m], WT [128c, 16s, 2048o]
  matmul: fp8 DoubleRow (c-chunk pairs on partitions), fp32 PSUM, N=512
     tiles, 8 PSUM banks, DVE drains, 512 KiB f32 output DMAs.

Schedule: w's first o-half + x group 0 load as c-ascending quads that the
first matmul group chases sp-pair by sp-pair; w's second o-half trickles
behind x groups 2-3; the o-half-1 matmul groups are emitted late, when
everything they need is already resident. Loads ride the two HWDGE FIFOs
(sync + scalar), stores ride the SWDGE queue; steady-state psum drains
stay exclusively on DVE so nothing can delay a drain (a blocked drain
holds a PSUM bank and stalls PE).
"""

import numpy as np
import ml_dtypes

# ---- problem constants (hardcoded per task contract) ----
A_DIM, B_DIM, C_DIM, OUT_DIM = 4, 2048, 2048, 8192
M_FULL = A_DIM * B_DIM  # 8192
GRID_M, GRID_O = 2, 4
N_CORES = GRID_M * GRID_O
M_CORE = M_FULL // GRID_M   # 4096
O_CORE = OUT_DIM // GRID_O  # 2048

P = 128
S_CHUNKS = C_DIM // P  # 16


def build_nc(m_core=M_CORE, o_core=O_CORE, c_dim=C_DIM,
             m_slab=512, n_tile=512, mm_psum_bufs=8):
    """Build the single-core Bass program (same program runs SPMD on 8 cores)."""
    import contextlib

    import concourse.bacc as bacc
    import concourse.mybir as mybir
    import concourse.tile as tile

    bf16 = mybir.dt.bfloat16
    f32 = mybir.dt.float32
    fp8 = mybir.dt.float8e4
    Copy = mybir.ActivationFunctionType.Copy
    DR = mybir.MatmulPerfMode.DoubleRow

    S = c_dim // P              # c-chunks (16)
    SP = S // 2                 # DoubleRow pairs (8)
    Q = S // 4                  # c-chunk quads (4)
    MG = m_core // m_slab       # x slab groups (8)
    MWG = m_slab // P           # m windows per slab group (4)
    NT = o_core // n_tile       # o tiles (4)
    NTH = NT // 2               # o tiles per half (2)
    o_half = o_core // 2        # 1024

    nc = bacc.Bacc(None, target_bir_lowering=False, debug=False)
    x_in = nc.declare_dram_parameter("x_in", [P, S, m_core], bf16, isOutput=False)
    w_in = nc.declare_dram_parameter("w_in", [P, S, o_core], bf16, isOutput=False)
    out = nc.declare_dram_parameter("out", [m_core, o_core], f32, isOutput=True)

    with tile.TileContext(nc) as tc:
        with contextlib.ExitStack() as ctx:
            wq = ctx.enter_context(tc.tile_pool(name="wq", bufs=3))
            xq = ctx.enter_context(tc.tile_pool(name="xq", bufs=8))
            xres = ctx.enter_context(tc.tile_pool(name="xres", bufs=1))
            wres = ctx.enter_context(tc.tile_pool(name="wres", bufs=1))
            mmp = ctx.enter_context(
                tc.tile_pool(name="mmp", bufs=mm_psum_bufs, space="PSUM"))
            osb = ctx.enter_context(tc.tile_pool(name="osb", bufs=4))

            # resident fp8 operands, c on partitions
            XT = xres.tile([P, S, m_core], fp8)
            WT = wres.tile([P, S, o_core], fp8)

            def w_seg(c0, nch, oh, qsel, qeng):
                """w c-chunks [c0..c0+nch) x o-half oh: one load + quant."""
                o0 = oh * o_half
                wst = wq.tile([P, 4, o_half], bf16, tag="wq", name="wq")
                src = w_in[:, c0:c0 + nch, o0:o0 + o_half]
                dst_st = wst[:, 0:nch, :]
                if qsel == 0:
                    nc.sync.dma_start(out=dst_st, in_=src)
                else:
                    nc.scalar.dma_start(out=dst_st, in_=src)
                dst = WT[:, c0:c0 + nch, o0:o0 + o_half]
                if qeng == 0:
                    nc.scalar.activation(dst, dst_st, Copy)
                else:
                    nc.vector.tensor_copy(out=dst, in_=dst_st)

            def x_seg(c0, nch, mg, qsel, qeng):
                """x c-chunks [c0..c0+nch) x m slab mg: one load + quant.
                Multi-chunk batching amortizes per-DMA fixed cost while the
                chase stays fine-grained (a chunk pair feeds one sp step)."""
                m0 = mg * m_slab
                xst = xq.tile([P, 4, m_slab], bf16, tag="xq", name="xq")
                src = x_in[:, c0:c0 + nch, m0:m0 + m_slab]
                dst_st = xst[:, 0:nch, :]
                if qsel == 0:
                    nc.sync.dma_start(out=dst_st, in_=src)
                else:
                    nc.scalar.dma_start(out=dst_st, in_=src)
                dst = XT[:, c0:c0 + nch, m0:m0 + m_slab]
                if qeng == 0:
                    nc.scalar.activation(dst, dst_st, Copy)
                else:
                    nc.vector.tensor_copy(out=dst, in_=dst_st)

            def w_quad(q, oh, qsel, qeng):
                w_seg(4 * q, 4, oh, qsel, qeng)

            def x_quad(q, mg, qsel, qeng):
                x_seg(4 * q, 4, mg, qsel, qeng)

            def x_wave(mg, qsel):
                """One x slab group as 4 pipelined quads (load q+1 overlaps
                quant q); a single 2 MiB DMA + monolithic quant was measured
                slower — its serial end-to-end latency stalls the chase."""
                for q in range(Q):
                    x_quad(q, mg, qsel=(q + qsel) % 2, qeng=0)

            def mm_half(mw, oh, tail=False):
                """One 128-row m window x one 1024-col o half: 8 sp x 2 nt
                DoubleRow matmuls, drain 2 psum tiles, 512 KiB store.

                Steady state keeps all drains on DVE (so nothing can delay a
                drain and hold a PSUM bank) and all stores on SWDGE. In the
                tail, ACT and the scalar HWDGE queue are idle, so drains and
                stores spread across both to shorten the endgame."""
                ps = [mmp.tile([P, n_tile], f32, tag="mm_psum", name="mm_psum")
                      for _ in range(NTH)]
                for sp in range(SP):
                    lhsT = XT[:, 2 * sp:2 * sp + 2, mw * P:(mw + 1) * P]
                    for j in range(NTH):
                        nt = NTH * oh + j
                        nc.tensor.matmul(
                            ps[j][:], lhsT,
                            WT[:, 2 * sp:2 * sp + 2, nt * n_tile:(nt + 1) * n_tile],
                            start=(sp == 0), stop=(sp == SP - 1),
                            perf_mode=DR)
                ot = osb.tile([P, o_half], f32, tag="ot", name="ot")
                for j in range(NTH):
                    dst = ot[:, j * n_tile:(j + 1) * n_tile]
                    if tail and j % 2 == 0:
                        nc.scalar.activation(dst, ps[j][:], Copy)
                    else:
                        nc.vector.tensor_copy(out=dst, in_=ps[j][:])
                odst = out[mw * P:(mw + 1) * P, oh * o_half:(oh + 1) * o_half]
                if tail and mw % 2 == 0:
                    nc.scalar.dma_start(out=odst, in_=ot[:])
                else:
                    nc.gpsimd.dma_start(out=odst, in_=ot[:])

            def mg_windows(mg):
                return range(mg * MWG, (mg + 1) * MWG)

            # ---- prefix: segments of (w o-half 0, x group 0) c-ascending on
            # both HWDGE queues, quants split ACT/DVE (safe ONLY here — no
            # drains exist yet); the first two segments are c-PAIRS so the
            # first sp starts ~4us; group 0 chases them sp-pair by sp-pair ----
            segs = [(0, 2), (2, 2), (4, 4), (8, 4), (12, 4)]
            for i, (c0, nch) in enumerate(segs):
                w_seg(c0, nch, 0, qsel=i % 2, qeng=i % 2)
                x_seg(c0, nch, 0, qsel=(i + 1) % 2, qeng=(i + 1) % 2)
            # prefetch wave for mg1 (quads, ACT quants) before group 0's
            # matmuls so its quants process during group 0's stream
            for q in range(Q):
                x_quad(q, 1, qsel=q % 2, qeng=0)
            for mw in mg_windows(0):
                mm_half(mw, 0)

            # ---- w's second o-half trickles behind each wave's x quads ----
            x_wave(2, qsel=0)
            w_quad(0, 1, qsel=0, qeng=1)
            w_quad(1, 1, qsel=1, qeng=1)
            for mw in mg_windows(1):
                mm_half(mw, 0)

            x_wave(3, qsel=1)
            w_quad(2, 1, qsel=0, qeng=1)
            w_quad(3, 1, qsel=1, qeng=1)
            for mw in mg_windows(2):
                mm_half(mw, 0)

            for mg in range(3, MG):
                if mg + 1 < MG:
                    x_wave(mg + 1, qsel=mg % 2)
                for mw in mg_windows(mg):
                    mm_half(mw, 0)
                if mg >= 4:
                    for mw in mg_windows(mg - 4):
                        mm_half(mw, 1)
            for mg in range(MG - 4, MG):
                for mw in mg_windows(mg):
                    mm_half(mw, 1, tail=(mg >= MG - 2))

    nc.finalize()
    return nc


def _round_to_odd_bf16(a):
    """f32 -> bf16 by truncation with the sticky bit ORed into the lsb.

    RNE(result -> e4m3) == RNE(a -> e4m3) exactly (no double rounding).
    """
    u = np.ascontiguousarray(a, dtype=np.float32).view(np.uint32)
    hi = (u >> 16).astype(np.uint16)
    hi |= ((u & 0xFFFF) != 0).astype(np.uint16)
    return hi.view(ml_dtypes.bfloat16)


def _stage(mat_t):
    """[c, free] -> [128, 16, free] with (p, s, f) = mat_t[s*128+p, f]."""
    c, free = mat_t.shape
    return np.ascontiguousarray(
        mat_t.reshape(S_CHUNKS, P, free).transpose(1, 0, 2))


_NC = None


def _get_nc():
    global _NC
    if _NC is None:
        _NC = build_nc()
    return _NC


def kernel(input, weight, input_scale_e4m3=None, weight_scale_e4m3=None,
           **_unused):
    from concourse.bass_utils import run_bass_kernel_spmd

    x = np.asarray(input, dtype=np.float32).reshape(M_FULL, C_DIM)
    w = np.asarray(weight, dtype=np.float32)
    s_in = float(np.asarray(input_scale_e4m3)) if input_scale_e4m3 is not None else 1.0
    s_w = float(np.asarray(weight_scale_e4m3)) if weight_scale_e4m3 is not None else 1.0

    # reference semantics: round(x*s)/s etc.; fold scales on host (exact)
    if s_in != 1.0:
        x = x * s_in
    if s_w != 1.0:
        w = w * s_w

    # host-side staging: round-to-odd bf16 (exact w.r.t. the later fp8
    # RNE quantize), transpose so c is the contraction/partition dim, and
    # pre-tile to [partition, c-chunk, free] for batched multi-chunk DMAs
    xb = _round_to_odd_bf16(x)
    wb = _round_to_odd_bf16(w)
    xT = [_stage(xb[mi * M_CORE:(mi + 1) * M_CORE].T)
          for mi in range(GRID_M)]
    wT = [_stage(wb[oj * O_CORE:(oj + 1) * O_CORE].T)
          for oj in range(GRID_O)]

    nc = _get_nc()
    in_maps = []
    for mi in range(GRID_M):
        for oj in range(GRID_O):
            in_maps.append({"x_in": xT[mi], "w_in": wT[oj]})
    res = run_bass_kernel_spmd(nc, in_maps, core_ids=list(range(N_CORES)))

    out = np.empty((M_FULL, OUT_DIM), np.float32)
    for k, r in enumerate(res.results):
        mi, oj = divmod(k, GRID_O)
        out[mi * M_CORE:(mi + 1) * M_CORE, oj * O_CORE:(oj + 1) * O_CORE] = r["out"]

    inv = 1.0 / (s_in * s_w)
    if inv != 1.0:
        out = out * inv
    return out.reshape(A_DIM, B_DIM, OUT_DIM)
